# revision 31
# baseline (speedup 1.0000x reference)
"""Causal self-attention TRN2 Bass kernel (interleaved-pipeline version).

Problem: B=4, T=2048, C=1024, H=16 heads (HD=64), torch-Linear semantics
(y = x @ W.T + b), causal + padding mask, softmax, output projection.

Sharding: 8 cores = (batch b in 0..3) x (head-half in 0..1). Each core
handles one batch and 8 heads (512 of the 1024 channels of QKV / of the
contraction dim of the output projection). The two half-cores of a batch
produce partial output projections that the host sums (plus bp).

Per-core schedule: a single software-pipelined stream of 4 flash-style
causal attention blocks in order qc = 0, 1, 3, 2. Between attention
steps the PE also executes injected background matmul groups: QKV
projections of later chunks and output projections of already-finished
q-chunks, paced to fill PE idle slots while the ACT engine streams the
exp()s. Q/K/V and the attention probabilities E are fp16 (full PE rate,
more mantissa than bf16, half the SBUF of f32 -- the E-tile ring depth
D=7 absorbs ACT jitter before the O-matmuls need each tile).

Engine assignment (engines are in-order; a multi-us op in front of a
PE-gating drain stalls the PE, so): Q/K/V PSUM drains and the output
staging copies run on the Scalar engine (Identity activation with
fused scale+bias); the softmax epilogue (O-PSUM drain, [1,512]
reciprocals, Y^T multiplies) runs on DVE; rowsum broadcast runs on
GpSimd (only that one op type -- mixing op types on GpSimd forces
~15us microcode swaps). Weight/x loads are a few large rearranged DMA
descriptors spread across the sync/scalar/gpsimd queues.
"""

import ml_dtypes
import numpy as np

import concourse.mybir as mybir
import concourse.tile as tile
from concourse import bacc
from concourse.bass_utils import run_bass_kernel_spmd

F32 = mybir.dt.float32
F32R = mybir.dt.float32r
BF16 = mybir.dt.bfloat16
FP16 = mybir.dt.float16
AF = mybir.ActivationFunctionType
ALU = mybir.AluOpType

B, T, C, H = 4, 2048, 1024, 16
HD = C // H          # 64
IC = C // 2          # 512 channels per core (8 heads)
NKT = T // 128       # 16 k-tiles
NCT = C // 128       # 8 contraction tiles for QKV
NEG = -1.0e30
SCALE = 1.0 / np.sqrt(HD)
D = 12               # O-matmul lag behind S/exp (pipeline depth)

_CACHE = {}


def _build():
    nc = bacc.Bacc("TRN2", target_bir_lowering=False, debug=False)

    xT_d = nc.dram_tensor("xT", [C, T], FP16, kind="ExternalInput").ap()
    WqT_d = nc.dram_tensor("WqT", [C, IC], FP16, kind="ExternalInput").ap()
    WkT_d = nc.dram_tensor("WkT", [C, IC], FP16, kind="ExternalInput").ap()
    WvT_d = nc.dram_tensor("WvT", [C, IC], FP16, kind="ExternalInput").ap()
    WpT_d = nc.dram_tensor("WpT", [IC, C], FP16, kind="ExternalInput").ap()
    bq_d = nc.dram_tensor("bqs", [128, 4], F32, kind="ExternalInput").ap()
    bk_d = nc.dram_tensor("bks", [128, 4], F32, kind="ExternalInput").ap()
    bv_d = nc.dram_tensor("bvr", [1, IC], F32, kind="ExternalInput").ap()
    pad_d = nc.dram_tensor("padb", [128, NKT], F32, kind="ExternalInput").ap()
    ones128_d = nc.dram_tensor("ones128", [1, 128], F32, kind="ExternalInput").ap()
    ones8f_d = nc.dram_tensor("ones8", [128, NKT * 8], FP16, kind="ExternalInput").ap()
    ident_d = nc.dram_tensor("ident", [128, 128], BF16, kind="ExternalInput").ap()
    maskb_d = nc.dram_tensor("maskb", [128, 128], BF16, kind="ExternalInput").ap()
    out_d = nc.dram_tensor("out", [T, C], F32, kind="ExternalOutput").ap()

    with tile.TileContext(nc) as tc:
        with tc.tile_pool(name="pp", bufs=1) as pp:
            # ---------------- persistent SBUF state ----------------
            KTs = [pp.tile([128, 4 * 512], FP16, name=f"KT{c}")
                   for c in range(4)]                            # per chunk
            QTs = [pp.tile([128, 4 * 512], FP16, name=f"QT{s}")
                   for s in range(2)]                            # 2-chunk ring
            Vts = [pp.tile([128, 4 * 520], FP16, name=f"Vt{c}")
                   for c in range(4)]                            # [V|1] x 8 heads
            YTs = [pp.tile([128, 4 * 512], FP16, name=f"YT{s}")
                   for s in range(2)]                            # 2-chunk ring
            Wq_sb = pp.tile([128, NCT * 512], FP16, name="Wq_sb")
            Wk_sb = pp.tile([128, NCT * 512], FP16, name="Wk_sb")
            Wv_sb = pp.tile([128, NCT * 512], FP16, name="Wv_sb")
            Wp_sb = pp.tile([128, 4 * C], FP16, name="Wp_sb")
            bq_sb = pp.tile([128, 4], F32, name="bq_sb")
            bk_sb = pp.tile([128, 4], F32, name="bk_sb")
            bv_sb = pp.tile([1, IC], F32R, name="bv_sb")
            pad_sb = pp.tile([128, NKT], F32, name="pad_sb")
            ones128 = pp.tile([1, 128], F32R, name="ones128")
            mask_sb = pp.tile([128, 128], BF16, name="mask_sb")
            ident_sb = pp.tile([128, 128], BF16, name="ident_sb")
            rbA = pp.tile([64, 512], F32, name="rbA")
            rbB = pp.tile([64, 512], F32, name="rbB")
            rrs = [pp.tile([33, 512], F32, name=f"rr{i}") for i in range(2)]
            rB0s = [pp.tile([1, 512], F32, name=f"rB0{i}") for i in range(2)]
            nc.vector.memset(rrs[0][:], 1.0)
            nc.vector.memset(rrs[1][:], 1.0)

            # ---- startup DMAs: first Q-matmul inputs lead the queue ----
            nc.sync.dma_start(out=Wq_sb[:, 0:512],
                              in_=WqT_d[0:128, :])
            xs = tc.alloc_tile_pool(name="xs", bufs=2)
            xcs = [None, None]

            def load_x(ch, first=False):
                s = ch % 2
                xcs[s] = xs.tile([128, NCT * 512], FP16, name="xc", tag="xc")
                src = xT_d[:, ch * 512:(ch + 1) * 512]
                if first:
                    nc.sync.dma_start(
                        out=xcs[s][:, 0:512],
                        in_=src[0:128, :])
                    nc.scalar.dma_start(
                        out=xcs[s].rearrange("p (ct t) -> p ct t", ct=NCT)[:, 1:4, :],
                        in_=src[128:512, :].rearrange(
                            "(ct p) t -> p ct t", p=128))
                    nc.scalar.dma_start(
                        out=xcs[s].rearrange("p (ct t) -> p ct t", ct=NCT)[:, 4:, :],
                        in_=src[512:, :].rearrange(
                            "(ct p) t -> p ct t", p=128))
                else:
                    nc.sync.dma_start(
                        out=xcs[s].rearrange("p (ct t) -> p ct t", ct=NCT),
                        in_=src.rearrange("(ct p) t -> p ct t", p=128))

            load_x(0, first=True)
            nc.gpsimd.dma_start(
                out=Wq_sb.rearrange("p (ct t) -> p ct t", ct=NCT)[:, 1:4, :],
                in_=WqT_d[128:512, :].rearrange(
                    "(ct p) t -> p ct t", p=128))
            nc.gpsimd.dma_start(
                out=Wq_sb.rearrange("p (ct t) -> p ct t", ct=NCT)[:, 4:, :],
                in_=WqT_d[512:, :].rearrange(
                    "(ct p) t -> p ct t", p=128))
            nc.scalar.dma_start(
                out=Wk_sb.rearrange("p (ct t) -> p ct t", ct=NCT),
                in_=WkT_d.rearrange("(ct p) t -> p ct t", p=128))
            nc.sync.dma_start(
                out=Wv_sb.rearrange("p (ct t) -> p ct t", ct=NCT),
                in_=WvT_d.rearrange("(ct p) t -> p ct t", p=128))
            nc.sync.dma_start(out=bq_sb[:], in_=bq_d)
            nc.sync.dma_start(out=bk_sb[:], in_=bk_d)
            nc.sync.dma_start(out=bv_sb[:], in_=bv_d.bitcast(F32R))
            nc.sync.dma_start(out=pad_sb[:], in_=pad_d)
            nc.sync.dma_start(out=ones128[:], in_=ones128_d.bitcast(F32R))
            nc.sync.dma_start(out=mask_sb[:], in_=maskb_d)
            nc.sync.dma_start(out=ident_sb[:], in_=ident_d)
            load_x(1)
            nc.sync.dma_start(
                out=Wp_sb.rearrange("p (g t) -> p g t", g=4),
                in_=WpT_d.rearrange("(g p) t -> p g t", p=128))
            Vrc = [Vts[c].rearrange("p (k h c) -> p k h c", k=4, h=8, c=65)
                   for c in range(4)]
            for c in range(4):
                nc.sync.dma_start(
                    out=Vrc[c][:, :, :, 64],
                    in_=ones8f_d[:, c * 32:(c + 1) * 32]
                    .rearrange("p (k h) -> p k h", k=4))

            # ---------------- pools ----------------
            es = tc.alloc_tile_pool(name="es", bufs=D + 1)
            osb = tc.alloc_tile_pool(name="osb", bufs=5)
            obp = tc.alloc_tile_pool(name="obp", bufs=3)
            pss = tc.alloc_tile_pool(name="pss", bufs=2, space="PSUM")
            pop = tc.alloc_tile_pool(name="pop", bufs=2, space="PSUM")
            inj = tc.alloc_tile_pool(name="inj", bufs=2, space="PSUM")

            # -------- background group generators (PE + ACT drains) --------
            def q_group(ch, g):
                def run():
                    s = ch % 2
                    pq = inj.tile([128, 512], F32, name="pq", tag="inj")
                    for ct in range(NCT):
                        nc.tensor.matmul(
                            out=pq[:],
                            lhsT=Wq_sb[:, ct * 512 + g * 128: ct * 512 + (g + 1) * 128],
                            rhs=xcs[s][:, ct * 512:(ct + 1) * 512],
                            start=(ct == 0), stop=(ct == NCT - 1),
                        )
                    nc.scalar.activation(
                        QTs[s][:, g * 512: (g + 1) * 512], pq[:], AF.Identity,
                        bias=bq_sb[:, g:g + 1], scale=SCALE)
                return run

            def k_group(ch, g):
                def run():
                    s = ch % 2
                    pk = inj.tile([128, 512], F32, name="pk", tag="inj")
                    for ct in range(NCT):
                        nc.tensor.matmul(
                            out=pk[:],
                            lhsT=Wk_sb[:, ct * 512 + g * 128: ct * 512 + (g + 1) * 128],
                            rhs=xcs[s][:, ct * 512:(ct + 1) * 512],
                            start=(ct == 0), stop=(ct == NCT - 1),
                        )
                    nc.scalar.activation(
                        KTs[ch][:, g * 512: g * 512 + 512], pk[:], AF.Identity,
                        bias=bk_sb[:, g:g + 1], scale=1.0)
                return run

            def v_group(ch, ts):
                def run():
                    s = ch % 2
                    kt = ch * 4 + ts
                    pv = inj.tile([128, 512], F32, name="pv", tag="inj")
                    for ct in range(NCT):
                        nc.tensor.matmul(
                            out=pv[:],
                            lhsT=xcs[s][:, ct * 512 + ts * 128: ct * 512 + ts * 128 + 128],
                            rhs=Wv_sb[:, ct * 512:(ct + 1) * 512],
                            start=(ct == 0), stop=False,
                        )
                    nc.tensor.matmul(
                        out=pv[:], lhsT=ones128[:], rhs=bv_sb[:],
                        start=False, stop=True,
                    )
                    nc.scalar.copy(Vrc[ch][:, ts, :, 0:64], pv[:])
                return run

            def proj_group(qc, tloc):
                def run():
                    s = qc % 2
                    tt = 4 * qc + tloc
                    for oc in range(2):
                        po = inj.tile([128, 512], F32, name="po", tag="inj")
                        for g in range(4):
                            nc.tensor.matmul(
                                out=po[:],
                                lhsT=YTs[s][:, g * 512 + tloc * 128:
                                            g * 512 + tloc * 128 + 128],
                                rhs=Wp_sb[:, g * C + oc * 512: g * C + oc * 512 + 512],
                                start=(g == 0), stop=(g == 3),
                            )
                        ob = obp.tile([128, 512], F32, name="ob", tag="ob")
                        nc.scalar.copy(ob[:], po[:])
                        nc.sync.dma_start(
                            out=out_d[tt * 128:(tt + 1) * 128,
                                      oc * 512:(oc + 1) * 512], in_=ob[:])
                return run

            def qk_groups(ch):
                gs = []
                for g in range(4):
                    gs.append(q_group(ch, g))
                    gs.append(k_group(ch, g))
                return gs

            def v_groups(ch):
                return [v_group(ch, ts) for ts in range(4)]

            # ---------------- attention for one (qc, g) ----------------
            def attention_g(qc, g, pump):
                s = qc % 2
                kmax = 4 * qc + 4
                oA = pop.tile([65, 512], F32, name="oA", tag="o")
                oB = pop.tile([65, 512], F32, name="oB", tag="o")
                e_l = [None] * kmax
                off_l = [None] * kmax
                for step in range(kmax + D):
                    if step < kmax:
                        kt = step
                        k0 = kt * 128
                        toff = 128 * (kt - 4 * qc) if kt >= 4 * qc else 0
                        off_l[kt] = toff
                        diag = kt >= 4 * qc
                        sAB = pss.tile([128, 1024], F32, name="sAB", tag="sAB")
                        kch, kloc = kt // 4, kt % 4
                        nc.tensor.matmul(
                            out=sAB[:, toff:512],
                            lhsT=KTs[kch][0:64, g * 512 + kloc * 128:
                                          g * 512 + kloc * 128 + 128],
                            rhs=QTs[s][0:64, g * 512 + toff: (g + 1) * 512],
                            start=True, stop=not diag,
                        )
                        nc.tensor.matmul(
                            out=sAB[:, 512 + toff:1024],
                            lhsT=KTs[kch][64:128, g * 512 + kloc * 128:
                                          g * 512 + kloc * 128 + 128],
                            rhs=QTs[s][64:128, g * 512 + toff: (g + 1) * 512],
                            start=True, stop=not diag, tile_position=(64, 0),
                        )
                        if diag:
                            nc.tensor.matmul(
                                out=sAB[:, toff:toff + 128],
                                lhsT=ident_sb[:], rhs=mask_sb[:],
                                start=False, stop=True,
                            )
                            nc.tensor.matmul(
                                out=sAB[:, 512 + toff:512 + toff + 128],
                                lhsT=ident_sb[:], rhs=mask_sb[:],
                                start=False, stop=True,
                            )
                        eAB = es.tile([128, 1024], FP16, name="eAB", tag="eAB")
                        s3 = sAB.rearrange("p (h w) -> p h w", h=2, w=512)
                        e3 = eAB.rearrange("p (h w) -> p h w", h=2, w=512)
                        nc.scalar.activation(
                            e3[:, :, toff:512], s3[:, :, toff:512], AF.Exp,
                            bias=pad_sb[:, kt:kt + 1])
                        e_l[kt] = eAB
                    pv = step - D
                    if 0 <= pv < kmax:
                        toff = off_l[pv]
                        vch, vbase = pv // 4, (pv % 4) * 520
                        nc.tensor.matmul(
                            out=oA[:, toff:512],
                            lhsT=Vts[vch][:, vbase + 130 * g: vbase + 130 * g + 65],
                            rhs=e_l[pv][:, toff:512],
                            start=(pv == 0), stop=(pv == kmax - 1),
                        )
                        nc.tensor.matmul(
                            out=oB[:, toff:512],
                            lhsT=Vts[vch][:, vbase + 130 * g + 65: vbase + 130 * g + 130],
                            rhs=e_l[pv][:, 512 + toff:1024],
                            start=(pv == 0), stop=(pv == kmax - 1),
                        )
                    pump()
                # epilogue: drain PSUM fast on DVE, normalize off-path
                oa_sb = osb.tile([65, 512], F32, name="oa_sb", tag="osb")
                ob_sb = osb.tile([65, 512], F32, name="ob_sb", tag="osb")
                nc.vector.tensor_copy(oa_sb[:], oA[:])
                nc.vector.tensor_copy(ob_sb[:], oB[:])
                rr = rrs[(qc * 4 + g) % 2]
                rB0 = rB0s[(qc * 4 + g) % 2]
                nc.vector.tensor_copy(rr[0:1, :], oa_sb[64:65, :])
                nc.vector.tensor_copy(rr[32:33, :], ob_sb[64:65, :])
                nc.vector.reciprocal(rr[:], rr[:])
                nc.vector.tensor_copy(rB0[:], rr[32:33, :])
                nc.gpsimd.partition_broadcast(rbA[:], rr[0:1, :])
                nc.gpsimd.partition_broadcast(rbB[:], rB0[:])
                nc.vector.tensor_mul(
                    YTs[s][0:64, g * 512: (g + 1) * 512],
                    oa_sb[0:64, :], rbA[:])
                nc.vector.tensor_mul(
                    YTs[s][64:128, g * 512: (g + 1) * 512],
                    ob_sb[0:64, :], rbB[:])

            # ---------------- the pipelined stream ----------------
            for grp in ([q_group(0, g) for g in range(4)]
                        + [k_group(0, g) for g in range(4)] + v_groups(0)):
                grp()

            order = [0, 1, 3, 2]
            for bi, qc in enumerate(order):
                if bi == 0:
                    load_x(2)
                elif bi == 1:
                    load_x(3)
                early, late, tail = [], [], []
                if qc == 0:
                    late = qk_groups(1) + v_groups(1)
                elif qc == 1:
                    # chunk-3 Q writes QTr slot 1, still read by this block's
                    # S matmuls -> those groups go to `tail`, emitted only
                    # after every qc=1 S matmul is in the stream
                    late = (qk_groups(2) + v_groups(2)
                            + [k_group(3, g) for g in range(4)] + v_groups(3))
                    tail = [q_group(3, g) for g in range(4)]
                elif qc == 3:
                    # proj(1) reads YTr slot 1 which this block's epilogues
                    # overwrite -> emit all proj(1) within g=0's steps
                    early = [proj_group(1, t) for t in range(4)]
                    late = [proj_group(0, t) for t in range(4)]
                else:  # qc == 2
                    late = [proj_group(3, t) for t in range(4)]

                g0_steps = 4 * qc + 4 + D
                nsteps = 4 * g0_steps
                stride_e = max(1, g0_steps // (len(early) + 1))
                stride_l = max(1, nsteps // (len(late) + 1))
                state = {"step": 0}

                def pump(state=state, early=early, late=late,
                         stride_e=stride_e, stride_l=stride_l):
                    state["step"] += 1
                    st = state["step"]
                    if early and st % stride_e == 0:
                        early.pop(0)()
                    elif late and st % stride_l == 0:
                        late.pop(0)()

                for g in range(4):
                    attention_g(qc, g, pump)
                    while early:
                        early.pop(0)()
                while late:
                    late.pop(0)()
                while tail:
                    tail.pop(0)()

            # tail: attention PSUM pools are dead -- use all 8 banks to
            # run proj(2)'s 8 chains g-major so only the final 8 matmuls
            # depend on the last epilogue
            inj.release()
            pop.release()
            pss.release()
            ptail = tc.alloc_tile_pool(name="ptail", bufs=1, space="PSUM")
            combos = [(tloc, oc) for tloc in range(4) for oc in range(2)]
            pos = [ptail.tile([128, 512], F32, name=f"pt{i}", tag=f"pt{i}")
                   for i in range(8)]
            for g in range(4):
                for i, (tloc, oc) in enumerate(combos):
                    nc.tensor.matmul(
                        out=pos[i][:],
                        lhsT=YTs[0][:, g * 512 + tloc * 128:
                                    g * 512 + tloc * 128 + 128],
                        rhs=Wp_sb[:, g * C + oc * 512: g * C + oc * 512 + 512],
                        start=(g == 0), stop=(g == 3),
                    )
            for i, (tloc, oc) in enumerate(combos):
                tt = 8 + tloc
                ob = obp.tile([128, 512], F32, name="ob", tag="ob")
                nc.scalar.copy(ob[:], pos[i][:])
                nc.sync.dma_start(
                    out=out_d[tt * 128:(tt + 1) * 128,
                              oc * 512:(oc + 1) * 512], in_=ob[:])

            for pool in (ptail, obp, osb, es, xs):
                pool.release()

    nc.compile()
    return nc


def _in_maps(x, Wk, bk, Wq, bq, Wv, bv, Wp, bp, padding_mask):
    maps = []
    mask_rows = np.arange(128)[:, None]
    mask_cols = np.arange(128)[None, :]
    maskb = np.where(mask_rows <= mask_cols, 0.0, NEG).astype(np.float32)
    for core in range(8):
        b, half = divmod(core, 2)
        hs = slice(half * IC, (half + 1) * IC)
        maps.append({
            "xT": np.ascontiguousarray(x[b].T).astype(np.float16),
            "WqT": np.ascontiguousarray(Wq[hs, :].T).astype(np.float16),
            "WkT": np.ascontiguousarray(Wk[hs, :].T).astype(np.float16),
            "WvT": np.ascontiguousarray(Wv[hs, :].T).astype(np.float16),
            "WpT": np.ascontiguousarray(Wp[:, hs].T).astype(np.float16),
            "bqs": np.ascontiguousarray((bq[hs] * SCALE).reshape(4, 128).T),
            "bks": np.ascontiguousarray(bk[hs].reshape(4, 128).T),
            "bvr": bv[hs].reshape(1, IC).copy(),
            "padb": np.ascontiguousarray(
                np.where(padding_mask[b] != 0, 0.0, NEG)
                .astype(np.float32).reshape(NKT, 128).T),
            "ones128": np.ones((1, 128), np.float32),
            "ones8": np.ones((128, NKT * 8), np.float16),
            "ident": np.eye(128).astype(ml_dtypes.bfloat16),
            "maskb": maskb.astype(ml_dtypes.bfloat16),
        })
    return maps


def _run(inputs, trace=False, **kw):
    if "nc" not in _CACHE:
        _CACHE["nc"] = _build()
    nc = _CACHE["nc"]
    ins = {k: np.asarray(v, dtype=np.float32) if k != "padding_mask"
           else np.asarray(v) for k, v in inputs.items()}
    maps = _in_maps(**ins)
    res = run_bass_kernel_spmd(nc, maps, core_ids=list(range(8)), trace=trace, **kw)
    bp = np.asarray(inputs["bp"], np.float32)
    y = np.empty((B, T, C), np.float32)
    for b in range(B):
        y[b] = res.results[2 * b]["out"] + res.results[2 * b + 1]["out"] + bp
    return y, res


def kernel(**inputs):
    y, _ = _run(inputs, trace=False)
    return y


# revision 33
# speedup vs baseline: 1.0486x; 1.0486x over previous
"""Causal self-attention TRN2 Bass kernel (interleaved-pipeline version).

Problem: B=4, T=2048, C=1024, H=16 heads (HD=64), torch-Linear semantics
(y = x @ W.T + b), causal + padding mask, softmax, output projection.

Sharding: 8 cores = (batch b in 0..3) x (head-half in 0..1). Each core
handles one batch and 8 heads (512 of the 1024 channels of QKV / of the
contraction dim of the output projection). The two half-cores of a batch
produce partial output projections that the host sums (plus bp).

Per-core schedule: a single software-pipelined stream of 4 flash-style
causal attention blocks in order qc = 0, 1, 3, 2. Between attention
steps the PE also executes injected background matmul groups: QKV
projections of later chunks and output projections of already-finished
q-chunks, paced to fill PE idle slots while the ACT engine streams the
exp()s. Q/K/V and the attention probabilities E are fp16 (full PE rate,
more mantissa than bf16, half the SBUF of f32 -- the E-tile ring depth
D=7 absorbs ACT jitter before the O-matmuls need each tile).

Engine assignment (engines are in-order; a multi-us op in front of a
PE-gating drain stalls the PE, so): Q/K/V PSUM drains and the output
staging copies run on the Scalar engine (Identity activation with
fused scale+bias); the softmax epilogue (O-PSUM drain, [1,512]
reciprocals, Y^T multiplies) runs on DVE; rowsum broadcast runs on
GpSimd (only that one op type -- mixing op types on GpSimd forces
~15us microcode swaps). Weight/x loads are a few large rearranged DMA
descriptors spread across the sync/scalar/gpsimd queues.
"""

import ml_dtypes
import numpy as np

import concourse.mybir as mybir
import concourse.tile as tile
from concourse import bacc
from concourse.bass_utils import run_bass_kernel_spmd

F32 = mybir.dt.float32
F32R = mybir.dt.float32r
BF16 = mybir.dt.bfloat16
FP16 = mybir.dt.float16
AF = mybir.ActivationFunctionType
ALU = mybir.AluOpType

B, T, C, H = 4, 2048, 1024, 16
HD = C // H          # 64
IC = C // 2          # 512 channels per core (8 heads)
NKT = T // 128       # 16 k-tiles
NCT = C // 128       # 8 contraction tiles for QKV
NEG = -1.0e30
SCALE = 1.0 / np.sqrt(HD)
D = 9                # O-matmul lag behind S/exp (pipeline depth)

_CACHE = {}


def _build():
    nc = bacc.Bacc("TRN2", target_bir_lowering=False, debug=False)

    xT_d = nc.dram_tensor("xT", [C, T], FP16, kind="ExternalInput").ap()
    WqT_d = nc.dram_tensor("WqT", [C, IC], FP16, kind="ExternalInput").ap()
    WkT_d = nc.dram_tensor("WkT", [C, IC], FP16, kind="ExternalInput").ap()
    WvT_d = nc.dram_tensor("WvT", [C, IC], FP16, kind="ExternalInput").ap()
    WpT_d = nc.dram_tensor("WpT", [IC, C], FP16, kind="ExternalInput").ap()
    bq_d = nc.dram_tensor("bqs", [128, 4], F32, kind="ExternalInput").ap()
    bk_d = nc.dram_tensor("bks", [128, 4], F32, kind="ExternalInput").ap()
    bv_d = nc.dram_tensor("bvr", [1, IC], F32, kind="ExternalInput").ap()
    pad_d = nc.dram_tensor("padb", [128, NKT], F32, kind="ExternalInput").ap()
    ones128_d = nc.dram_tensor("ones128", [1, 128], F32, kind="ExternalInput").ap()
    ones8f_d = nc.dram_tensor("ones8", [128, NKT * 8], FP16, kind="ExternalInput").ap()
    ident_d = nc.dram_tensor("ident", [128, 128], BF16, kind="ExternalInput").ap()
    maskb_d = nc.dram_tensor("maskb", [128, 128], BF16, kind="ExternalInput").ap()
    out_d = nc.dram_tensor("out", [T, C], F32, kind="ExternalOutput").ap()

    with tile.TileContext(nc) as tc:
        with tc.tile_pool(name="pp", bufs=1) as pp:
            # ---------------- persistent SBUF state ----------------
            KTs = [pp.tile([128, 4 * 512], FP16, name=f"KT{c}")
                   for c in range(4)]                            # per chunk
            QTs = [pp.tile([128, 4 * 512], FP16, name=f"QT{s}")
                   for s in range(2)]                            # 2-chunk ring
            Vts = [pp.tile([128, 4 * 520], FP16, name=f"Vt{c}")
                   for c in range(4)]                            # [V|1] x 8 heads
            YTs = [pp.tile([128, 4 * 512], FP16, name=f"YT{s}")
                   for s in range(2)]                            # 2-chunk ring
            Wq_sb = pp.tile([128, NCT * 512], FP16, name="Wq_sb")
            Wk_sb = pp.tile([128, NCT * 512], FP16, name="Wk_sb")
            Wv_sb = pp.tile([128, NCT * 512], FP16, name="Wv_sb")
            Wp_sb = pp.tile([128, 4 * C], FP16, name="Wp_sb")
            bq_sb = pp.tile([128, 4], F32, name="bq_sb")
            bk_sb = pp.tile([128, 4], F32, name="bk_sb")
            bv_sb = pp.tile([1, IC], F32R, name="bv_sb")
            pad_sb = pp.tile([128, NKT], F32, name="pad_sb")
            ones128 = pp.tile([1, 128], F32R, name="ones128")
            mask_sb = pp.tile([128, 128], BF16, name="mask_sb")
            ident_sb = pp.tile([128, 128], BF16, name="ident_sb")
            rbA = pp.tile([64, 512], F32, name="rbA")
            rbB = pp.tile([64, 512], F32, name="rbB")
            rrs = [pp.tile([33, 512], F32, name=f"rr{i}") for i in range(2)]
            rB0s = [pp.tile([1, 512], F32, name=f"rB0{i}") for i in range(2)]
            nc.vector.memset(rrs[0][:], 1.0)
            nc.vector.memset(rrs[1][:], 1.0)

            # ---- startup DMAs: first Q-matmul inputs lead the queue ----
            nc.sync.dma_start(out=Wq_sb[:, 0:512],
                              in_=WqT_d[0:128, :])
            xs = tc.alloc_tile_pool(name="xs", bufs=2)
            xcs = [None, None]

            def load_x(ch, first=False):
                s = ch % 2
                xcs[s] = xs.tile([128, NCT * 512], FP16, name="xc", tag="xc")
                src = xT_d[:, ch * 512:(ch + 1) * 512]
                if first:
                    nc.sync.dma_start(
                        out=xcs[s][:, 0:512],
                        in_=src[0:128, :])
                    nc.scalar.dma_start(
                        out=xcs[s].rearrange("p (ct t) -> p ct t", ct=NCT)[:, 1:4, :],
                        in_=src[128:512, :].rearrange(
                            "(ct p) t -> p ct t", p=128))
                    nc.scalar.dma_start(
                        out=xcs[s].rearrange("p (ct t) -> p ct t", ct=NCT)[:, 4:, :],
                        in_=src[512:, :].rearrange(
                            "(ct p) t -> p ct t", p=128))
                else:
                    nc.sync.dma_start(
                        out=xcs[s].rearrange("p (ct t) -> p ct t", ct=NCT),
                        in_=src.rearrange("(ct p) t -> p ct t", p=128))

            load_x(0, first=True)
            nc.gpsimd.dma_start(
                out=Wq_sb.rearrange("p (ct t) -> p ct t", ct=NCT)[:, 1:4, :],
                in_=WqT_d[128:512, :].rearrange(
                    "(ct p) t -> p ct t", p=128))
            nc.gpsimd.dma_start(
                out=Wq_sb.rearrange("p (ct t) -> p ct t", ct=NCT)[:, 4:, :],
                in_=WqT_d[512:, :].rearrange(
                    "(ct p) t -> p ct t", p=128))
            nc.scalar.dma_start(
                out=Wk_sb.rearrange("p (ct t) -> p ct t", ct=NCT),
                in_=WkT_d.rearrange("(ct p) t -> p ct t", p=128))
            nc.sync.dma_start(
                out=Wv_sb.rearrange("p (ct t) -> p ct t", ct=NCT),
                in_=WvT_d.rearrange("(ct p) t -> p ct t", p=128))
            nc.sync.dma_start(out=bq_sb[:], in_=bq_d)
            nc.sync.dma_start(out=bk_sb[:], in_=bk_d)
            nc.sync.dma_start(out=bv_sb[:], in_=bv_d.bitcast(F32R))
            nc.sync.dma_start(out=pad_sb[:], in_=pad_d)
            nc.sync.dma_start(out=ones128[:], in_=ones128_d.bitcast(F32R))
            nc.sync.dma_start(out=mask_sb[:], in_=maskb_d)
            nc.sync.dma_start(out=ident_sb[:], in_=ident_d)
            load_x(1)
            nc.sync.dma_start(
                out=Wp_sb.rearrange("p (g t) -> p g t", g=4),
                in_=WpT_d.rearrange("(g p) t -> p g t", p=128))
            Vrc = [Vts[c].rearrange("p (k h c) -> p k h c", k=4, h=8, c=65)
                   for c in range(4)]
            for c in range(4):
                nc.sync.dma_start(
                    out=Vrc[c][:, :, :, 64],
                    in_=ones8f_d[:, c * 32:(c + 1) * 32]
                    .rearrange("p (k h) -> p k h", k=4))

            # ---------------- pools ----------------
            es = tc.alloc_tile_pool(name="es", bufs=D + 1)
            osb = tc.alloc_tile_pool(name="osb", bufs=5)
            obp = tc.alloc_tile_pool(name="obp", bufs=3)
            pss = tc.alloc_tile_pool(name="pss", bufs=2, space="PSUM")
            pop = tc.alloc_tile_pool(name="pop", bufs=2, space="PSUM")
            inj = tc.alloc_tile_pool(name="inj", bufs=2, space="PSUM")

            # -------- background group generators (PE + ACT drains) --------
            def q_group(ch, g):
                def run():
                    s = ch % 2
                    pq = inj.tile([128, 512], F32, name="pq", tag="inj")
                    for ct in range(NCT):
                        nc.tensor.matmul(
                            out=pq[:],
                            lhsT=Wq_sb[:, ct * 512 + g * 128: ct * 512 + (g + 1) * 128],
                            rhs=xcs[s][:, ct * 512:(ct + 1) * 512],
                            start=(ct == 0), stop=(ct == NCT - 1),
                        )
                    nc.scalar.activation(
                        QTs[s][:, g * 512: (g + 1) * 512], pq[:], AF.Identity,
                        bias=bq_sb[:, g:g + 1], scale=SCALE)
                return run

            def k_group(ch, g):
                def run():
                    s = ch % 2
                    pk = inj.tile([128, 512], F32, name="pk", tag="inj")
                    for ct in range(NCT):
                        nc.tensor.matmul(
                            out=pk[:],
                            lhsT=Wk_sb[:, ct * 512 + g * 128: ct * 512 + (g + 1) * 128],
                            rhs=xcs[s][:, ct * 512:(ct + 1) * 512],
                            start=(ct == 0), stop=(ct == NCT - 1),
                        )
                    nc.scalar.activation(
                        KTs[ch][:, g * 512: g * 512 + 512], pk[:], AF.Identity,
                        bias=bk_sb[:, g:g + 1], scale=1.0)
                return run

            def v_group(ch, ts):
                def run():
                    s = ch % 2
                    kt = ch * 4 + ts
                    pv = inj.tile([128, 512], F32, name="pv", tag="inj")
                    for ct in range(NCT):
                        nc.tensor.matmul(
                            out=pv[:],
                            lhsT=xcs[s][:, ct * 512 + ts * 128: ct * 512 + ts * 128 + 128],
                            rhs=Wv_sb[:, ct * 512:(ct + 1) * 512],
                            start=(ct == 0), stop=False,
                        )
                    nc.tensor.matmul(
                        out=pv[:], lhsT=ones128[:], rhs=bv_sb[:],
                        start=False, stop=True,
                    )
                    nc.scalar.copy(Vrc[ch][:, ts, :, 0:64], pv[:])
                return run

            def proj_group(qc, tloc):
                def run():
                    s = qc % 2
                    tt = 4 * qc + tloc
                    for oc in range(2):
                        po = inj.tile([128, 512], F32, name="po", tag="inj")
                        for g in range(4):
                            nc.tensor.matmul(
                                out=po[:],
                                lhsT=YTs[s][:, g * 512 + tloc * 128:
                                            g * 512 + tloc * 128 + 128],
                                rhs=Wp_sb[:, g * C + oc * 512: g * C + oc * 512 + 512],
                                start=(g == 0), stop=(g == 3),
                            )
                        ob = obp.tile([128, 512], F32, name="ob", tag="ob")
                        if oc == 0:
                            nc.scalar.copy(ob[:], po[:])
                        else:
                            nc.vector.tensor_copy(ob[:], po[:])
                        nc.sync.dma_start(
                            out=out_d[tt * 128:(tt + 1) * 128,
                                      oc * 512:(oc + 1) * 512], in_=ob[:])
                return run

            def qk_groups(ch):
                gs = []
                for g in range(4):
                    gs.append(q_group(ch, g))
                    gs.append(k_group(ch, g))
                return gs

            def v_groups(ch):
                return [v_group(ch, ts) for ts in range(4)]

            # ---------------- attention for one (qc, g) ----------------
            def attention_g(qc, g, pump):
                s = qc % 2
                kmax = 4 * qc + 4
                oA = pop.tile([65, 512], F32, name="oA", tag="o")
                oB = pop.tile([65, 512], F32, name="oB", tag="o")
                e_l = [None] * kmax
                off_l = [None] * kmax
                for step in range(kmax + D):
                    if step < kmax:
                        kt = step
                        k0 = kt * 128
                        toff = 128 * (kt - 4 * qc) if kt >= 4 * qc else 0
                        off_l[kt] = toff
                        diag = kt >= 4 * qc
                        sAB = pss.tile([128, 1024], F32, name="sAB", tag="sAB")
                        kch, kloc = kt // 4, kt % 4
                        nc.tensor.matmul(
                            out=sAB[:, toff:512],
                            lhsT=KTs[kch][0:64, g * 512 + kloc * 128:
                                          g * 512 + kloc * 128 + 128],
                            rhs=QTs[s][0:64, g * 512 + toff: (g + 1) * 512],
                            start=True, stop=not diag,
                        )
                        nc.tensor.matmul(
                            out=sAB[:, 512 + toff:1024],
                            lhsT=KTs[kch][64:128, g * 512 + kloc * 128:
                                          g * 512 + kloc * 128 + 128],
                            rhs=QTs[s][64:128, g * 512 + toff: (g + 1) * 512],
                            start=True, stop=not diag, tile_position=(64, 0),
                        )
                        if diag:
                            nc.tensor.matmul(
                                out=sAB[:, toff:toff + 128],
                                lhsT=ident_sb[:], rhs=mask_sb[:],
                                start=False, stop=True,
                            )
                            nc.tensor.matmul(
                                out=sAB[:, 512 + toff:512 + toff + 128],
                                lhsT=ident_sb[:], rhs=mask_sb[:],
                                start=False, stop=True,
                            )
                        eAB = es.tile([128, 1024], FP16, name="eAB", tag="eAB")
                        s3 = sAB.rearrange("p (h w) -> p h w", h=2, w=512)
                        e3 = eAB.rearrange("p (h w) -> p h w", h=2, w=512)
                        nc.scalar.activation(
                            e3[:, :, toff:512], s3[:, :, toff:512], AF.Exp,
                            bias=pad_sb[:, kt:kt + 1])
                        e_l[kt] = eAB
                    pv = step - D
                    if 0 <= pv < kmax:
                        toff = off_l[pv]
                        vch, vbase = pv // 4, (pv % 4) * 520
                        nc.tensor.matmul(
                            out=oA[:, toff:512],
                            lhsT=Vts[vch][:, vbase + 130 * g: vbase + 130 * g + 65],
                            rhs=e_l[pv][:, toff:512],
                            start=(pv == 0), stop=(pv == kmax - 1),
                        )
                        nc.tensor.matmul(
                            out=oB[:, toff:512],
                            lhsT=Vts[vch][:, vbase + 130 * g + 65: vbase + 130 * g + 130],
                            rhs=e_l[pv][:, 512 + toff:1024],
                            start=(pv == 0), stop=(pv == kmax - 1),
                        )
                    pump()
                # epilogue: drain PSUM fast on DVE, normalize off-path
                oa_sb = osb.tile([65, 512], F32, name="oa_sb", tag="osb")
                ob_sb = osb.tile([65, 512], F32, name="ob_sb", tag="osb")
                nc.vector.tensor_copy(oa_sb[:], oA[:])
                nc.vector.tensor_copy(ob_sb[:], oB[:])
                rr = rrs[(qc * 4 + g) % 2]
                rB0 = rB0s[(qc * 4 + g) % 2]
                nc.vector.tensor_copy(rr[0:1, :], oa_sb[64:65, :])
                nc.vector.tensor_copy(rr[32:33, :], ob_sb[64:65, :])
                nc.vector.reciprocal(rr[:], rr[:])
                nc.vector.tensor_copy(rB0[:], rr[32:33, :])
                nc.gpsimd.partition_broadcast(rbA[:], rr[0:1, :])
                nc.gpsimd.partition_broadcast(rbB[:], rB0[:])
                nc.vector.tensor_mul(
                    YTs[s][0:64, g * 512: (g + 1) * 512],
                    oa_sb[0:64, :], rbA[:])
                nc.vector.tensor_mul(
                    YTs[s][64:128, g * 512: (g + 1) * 512],
                    ob_sb[0:64, :], rbB[:])

            # ---------------- the pipelined stream ----------------
            for grp in ([q_group(0, g) for g in range(4)]
                        + [k_group(0, g) for g in range(4)] + v_groups(0)):
                grp()

            order = [0, 1, 3, 2]
            for bi, qc in enumerate(order):
                if bi == 0:
                    load_x(2)
                elif bi == 1:
                    load_x(3)
                early, late, tail = [], [], []
                if qc == 0:
                    late = qk_groups(1) + v_groups(1)
                elif qc == 1:
                    # chunk-3 Q writes QTr slot 1, still read by this block's
                    # S matmuls -> those groups go to `tail`, emitted only
                    # after every qc=1 S matmul is in the stream
                    late = (qk_groups(2) + v_groups(2)
                            + [k_group(3, g) for g in range(4)] + v_groups(3))
                    tail = [q_group(3, g) for g in range(4)]
                elif qc == 3:
                    # proj(1) reads YTr slot 1 which this block's epilogues
                    # overwrite -> emit all proj(1) within g=0's steps
                    early = [proj_group(1, t) for t in range(4)]
                    late = [proj_group(0, t) for t in range(4)]
                else:  # qc == 2
                    late = [proj_group(3, t) for t in range(4)]

                g0_steps = 4 * qc + 4 + D
                nsteps = 4 * g0_steps
                stride_e = max(1, g0_steps // (len(early) + 1))
                stride_l = max(1, nsteps // (len(late) + 1))
                state = {"step": 0}

                def pump(state=state, early=early, late=late,
                         stride_e=stride_e, stride_l=stride_l):
                    state["step"] += 1
                    st = state["step"]
                    if early and st % stride_e == 0:
                        early.pop(0)()
                    elif late and st % stride_l == 0:
                        late.pop(0)()

                for g in range(4):
                    attention_g(qc, g, pump)
                    while early:
                        early.pop(0)()
                while late:
                    late.pop(0)()
                while tail:
                    tail.pop(0)()

            # tail: attention PSUM pools are dead -- use all 8 banks to
            # run proj(2)'s 8 chains g-major so only the final 8 matmuls
            # depend on the last epilogue
            inj.release()
            pop.release()
            pss.release()
            ptail = tc.alloc_tile_pool(name="ptail", bufs=1, space="PSUM")
            combos = [(tloc, oc) for tloc in range(4) for oc in range(2)]
            pos = [ptail.tile([128, 512], F32, name=f"pt{i}", tag=f"pt{i}")
                   for i in range(8)]
            for g in range(4):
                for i, (tloc, oc) in enumerate(combos):
                    nc.tensor.matmul(
                        out=pos[i][:],
                        lhsT=YTs[0][:, g * 512 + tloc * 128:
                                    g * 512 + tloc * 128 + 128],
                        rhs=Wp_sb[:, g * C + oc * 512: g * C + oc * 512 + 512],
                        start=(g == 0), stop=(g == 3),
                    )
            for i, (tloc, oc) in enumerate(combos):
                tt = 8 + tloc
                ob = obp.tile([128, 512], F32, name="ob", tag="ob")
                nc.scalar.copy(ob[:], pos[i][:])
                nc.sync.dma_start(
                    out=out_d[tt * 128:(tt + 1) * 128,
                              oc * 512:(oc + 1) * 512], in_=ob[:])

            for pool in (ptail, obp, osb, es, xs):
                pool.release()

    nc.compile()
    return nc


def _in_maps(x, Wk, bk, Wq, bq, Wv, bv, Wp, bp, padding_mask):
    maps = []
    mask_rows = np.arange(128)[:, None]
    mask_cols = np.arange(128)[None, :]
    maskb = np.where(mask_rows <= mask_cols, 0.0, NEG).astype(np.float32)
    for core in range(8):
        b, half = divmod(core, 2)
        hs = slice(half * IC, (half + 1) * IC)
        maps.append({
            "xT": np.ascontiguousarray(x[b].T).astype(np.float16),
            "WqT": np.ascontiguousarray(Wq[hs, :].T).astype(np.float16),
            "WkT": np.ascontiguousarray(Wk[hs, :].T).astype(np.float16),
            "WvT": np.ascontiguousarray(Wv[hs, :].T).astype(np.float16),
            "WpT": np.ascontiguousarray(Wp[:, hs].T).astype(np.float16),
            "bqs": np.ascontiguousarray((bq[hs] * SCALE).reshape(4, 128).T),
            "bks": np.ascontiguousarray(bk[hs].reshape(4, 128).T),
            "bvr": bv[hs].reshape(1, IC).copy(),
            "padb": np.ascontiguousarray(
                np.where(padding_mask[b] != 0, 0.0, NEG)
                .astype(np.float32).reshape(NKT, 128).T),
            "ones128": np.ones((1, 128), np.float32),
            "ones8": np.ones((128, NKT * 8), np.float16),
            "ident": np.eye(128).astype(ml_dtypes.bfloat16),
            "maskb": maskb.astype(ml_dtypes.bfloat16),
        })
    return maps


def _run(inputs, trace=False, **kw):
    if "nc" not in _CACHE:
        _CACHE["nc"] = _build()
    nc = _CACHE["nc"]
    ins = {k: np.asarray(v, dtype=np.float32) if k != "padding_mask"
           else np.asarray(v) for k, v in inputs.items()}
    maps = _in_maps(**ins)
    res = run_bass_kernel_spmd(nc, maps, core_ids=list(range(8)), trace=trace, **kw)
    bp = np.asarray(inputs["bp"], np.float32)
    y = np.empty((B, T, C), np.float32)
    for b in range(B):
        y[b] = res.results[2 * b]["out"] + res.results[2 * b + 1]["out"] + bp
    return y, res


def kernel(**inputs):
    y, _ = _run(inputs, trace=False)
    return y


# revision 34
# speedup vs baseline: 1.0541x; 1.0053x over previous
"""Causal self-attention TRN2 Bass kernel (interleaved-pipeline version).

Problem: B=4, T=2048, C=1024, H=16 heads (HD=64), torch-Linear semantics
(y = x @ W.T + b), causal + padding mask, softmax, output projection.

Sharding: 8 cores = (batch b in 0..3) x (head-half in 0..1). Each core
handles one batch and 8 heads (512 of the 1024 channels of QKV / of the
contraction dim of the output projection). The two half-cores of a batch
produce partial output projections that the host sums (plus bp).

Per-core schedule: a single software-pipelined stream of 4 flash-style
causal attention blocks in order qc = 0, 1, 3, 2. Between attention
steps the PE also executes injected background matmul groups: QKV
projections of later chunks and output projections of already-finished
q-chunks, paced to fill PE idle slots while the ACT engine streams the
exp()s. Q/K/V and the attention probabilities E are fp16 (full PE rate,
more mantissa than bf16, half the SBUF of f32 -- the E-tile ring depth
D=7 absorbs ACT jitter before the O-matmuls need each tile).

Engine assignment (engines are in-order; a multi-us op in front of a
PE-gating drain stalls the PE, so): Q/K/V PSUM drains and the output
staging copies run on the Scalar engine (Identity activation with
fused scale+bias); the softmax epilogue (O-PSUM drain, [1,512]
reciprocals, Y^T multiplies) runs on DVE; rowsum broadcast runs on
GpSimd (only that one op type -- mixing op types on GpSimd forces
~15us microcode swaps). Weight/x loads are a few large rearranged DMA
descriptors spread across the sync/scalar/gpsimd queues.
"""

import ml_dtypes
import numpy as np

import concourse.mybir as mybir
import concourse.tile as tile
from concourse import bacc
from concourse.bass_utils import run_bass_kernel_spmd

F32 = mybir.dt.float32
F32R = mybir.dt.float32r
BF16 = mybir.dt.bfloat16
FP16 = mybir.dt.float16
AF = mybir.ActivationFunctionType
ALU = mybir.AluOpType

B, T, C, H = 4, 2048, 1024, 16
HD = C // H          # 64
IC = C // 2          # 512 channels per core (8 heads)
NKT = T // 128       # 16 k-tiles
NCT = C // 128       # 8 contraction tiles for QKV
NEG = -1.0e30
SCALE = 1.0 / np.sqrt(HD)
D = 9                # O-matmul lag behind S/exp (pipeline depth)

_CACHE = {}


def _build():
    nc = bacc.Bacc("TRN2", target_bir_lowering=False, debug=False)

    xT_d = nc.dram_tensor("xT", [C, T], FP16, kind="ExternalInput").ap()
    WqT_d = nc.dram_tensor("WqT", [C, IC], FP16, kind="ExternalInput").ap()
    WkT_d = nc.dram_tensor("WkT", [C, IC], FP16, kind="ExternalInput").ap()
    WvT_d = nc.dram_tensor("WvT", [C, IC], FP16, kind="ExternalInput").ap()
    WpT_d = nc.dram_tensor("WpT", [IC, C], FP16, kind="ExternalInput").ap()
    bq_d = nc.dram_tensor("bqs", [128, 4], F32, kind="ExternalInput").ap()
    bk_d = nc.dram_tensor("bks", [128, 4], F32, kind="ExternalInput").ap()
    bv_d = nc.dram_tensor("bvr", [1, IC], F32, kind="ExternalInput").ap()
    pad_d = nc.dram_tensor("padb", [128, NKT], F32, kind="ExternalInput").ap()
    ones128_d = nc.dram_tensor("ones128", [1, 128], F32, kind="ExternalInput").ap()
    ones8f_d = nc.dram_tensor("ones8", [128, NKT * 8], FP16, kind="ExternalInput").ap()
    ident_d = nc.dram_tensor("ident", [128, 128], BF16, kind="ExternalInput").ap()
    maskb_d = nc.dram_tensor("maskb", [128, 128], BF16, kind="ExternalInput").ap()
    out_d = nc.dram_tensor("out", [T, C], F32, kind="ExternalOutput").ap()

    with tile.TileContext(nc) as tc:
        with tc.tile_pool(name="pp", bufs=1) as pp:
            # ---------------- persistent SBUF state ----------------
            KTs = [pp.tile([128, 4 * 512], FP16, name=f"KT{c}")
                   for c in range(4)]                            # per chunk
            QTs = [pp.tile([128, 4 * 512], FP16, name=f"QT{s}")
                   for s in range(2)]                            # 2-chunk ring
            Vts = [pp.tile([128, 4 * 520], FP16, name=f"Vt{c}")
                   for c in range(4)]                            # [V|1] x 8 heads
            YTs = [pp.tile([128, 4 * 512], FP16, name=f"YT{s}")
                   for s in range(2)]                            # 2-chunk ring
            Wq_sb = pp.tile([128, NCT * 512], FP16, name="Wq_sb")
            Wk_sb = pp.tile([128, NCT * 512], FP16, name="Wk_sb")
            Wv_sb = pp.tile([128, NCT * 512], FP16, name="Wv_sb")
            Wp_sb = pp.tile([128, 4 * C], FP16, name="Wp_sb")
            bq_sb = pp.tile([128, 4], F32, name="bq_sb")
            bk_sb = pp.tile([128, 4], F32, name="bk_sb")
            bv_sb = pp.tile([1, IC], F32R, name="bv_sb")
            pad_sb = pp.tile([128, NKT], F32, name="pad_sb")
            ones128 = pp.tile([1, 128], F32R, name="ones128")
            mask_sb = pp.tile([128, 128], BF16, name="mask_sb")
            ident_sb = pp.tile([128, 128], BF16, name="ident_sb")
            rbA = pp.tile([64, 512], F32, name="rbA")
            rbB = pp.tile([64, 512], F32, name="rbB")
            rrs = [pp.tile([33, 512], F32, name=f"rr{i}") for i in range(2)]
            rB0s = [pp.tile([1, 512], F32, name=f"rB0{i}") for i in range(2)]
            nc.vector.memset(rrs[0][:], 1.0)
            nc.vector.memset(rrs[1][:], 1.0)

            # ---- startup DMAs: first Q-matmul inputs lead the queue ----
            nc.sync.dma_start(out=Wq_sb[:, 0:512],
                              in_=WqT_d[0:128, :])
            xs = tc.alloc_tile_pool(name="xs", bufs=2)
            xcs = [None, None]

            def load_x(ch, first=False):
                s = ch % 2
                xcs[s] = xs.tile([128, NCT * 512], FP16, name="xc", tag="xc")
                src = xT_d[:, ch * 512:(ch + 1) * 512]
                if first:
                    nc.sync.dma_start(
                        out=xcs[s][:, 0:512],
                        in_=src[0:128, :])
                    nc.scalar.dma_start(
                        out=xcs[s].rearrange("p (ct t) -> p ct t", ct=NCT)[:, 1:4, :],
                        in_=src[128:512, :].rearrange(
                            "(ct p) t -> p ct t", p=128))
                    nc.scalar.dma_start(
                        out=xcs[s].rearrange("p (ct t) -> p ct t", ct=NCT)[:, 4:, :],
                        in_=src[512:, :].rearrange(
                            "(ct p) t -> p ct t", p=128))
                else:
                    nc.sync.dma_start(
                        out=xcs[s].rearrange("p (ct t) -> p ct t", ct=NCT),
                        in_=src.rearrange("(ct p) t -> p ct t", p=128))

            load_x(0, first=True)
            nc.gpsimd.dma_start(
                out=Wq_sb.rearrange("p (ct t) -> p ct t", ct=NCT)[:, 1:4, :],
                in_=WqT_d[128:512, :].rearrange(
                    "(ct p) t -> p ct t", p=128))
            nc.gpsimd.dma_start(
                out=Wq_sb.rearrange("p (ct t) -> p ct t", ct=NCT)[:, 4:, :],
                in_=WqT_d[512:, :].rearrange(
                    "(ct p) t -> p ct t", p=128))
            nc.scalar.dma_start(
                out=Wk_sb.rearrange("p (ct t) -> p ct t", ct=NCT),
                in_=WkT_d.rearrange("(ct p) t -> p ct t", p=128))
            nc.sync.dma_start(
                out=Wv_sb.rearrange("p (ct t) -> p ct t", ct=NCT),
                in_=WvT_d.rearrange("(ct p) t -> p ct t", p=128))
            nc.sync.dma_start(out=bq_sb[:], in_=bq_d)
            nc.sync.dma_start(out=bk_sb[:], in_=bk_d)
            nc.sync.dma_start(out=bv_sb[:], in_=bv_d.bitcast(F32R))
            nc.sync.dma_start(out=pad_sb[:], in_=pad_d)
            nc.sync.dma_start(out=ones128[:], in_=ones128_d.bitcast(F32R))
            nc.sync.dma_start(out=mask_sb[:], in_=maskb_d)
            nc.sync.dma_start(out=ident_sb[:], in_=ident_d)
            load_x(1)
            nc.sync.dma_start(
                out=Wp_sb.rearrange("p (g t) -> p g t", g=4),
                in_=WpT_d.rearrange("(g p) t -> p g t", p=128))
            Vrc = [Vts[c].rearrange("p (k h c) -> p k h c", k=4, h=8, c=65)
                   for c in range(4)]
            for c in range(4):
                nc.sync.dma_start(
                    out=Vrc[c][:, :, :, 64],
                    in_=ones8f_d[:, c * 32:(c + 1) * 32]
                    .rearrange("p (k h) -> p k h", k=4))

            # ---------------- pools ----------------
            es = tc.alloc_tile_pool(name="es", bufs=D + 1)
            osb = tc.alloc_tile_pool(name="osb", bufs=6)
            obp = tc.alloc_tile_pool(name="obp", bufs=4)
            pss = tc.alloc_tile_pool(name="pss", bufs=2, space="PSUM")
            pop = tc.alloc_tile_pool(name="pop", bufs=2, space="PSUM")
            inj = tc.alloc_tile_pool(name="inj", bufs=2, space="PSUM")

            # -------- background group generators (PE + ACT drains) --------
            def q_group(ch, g):
                def run():
                    s = ch % 2
                    pq = inj.tile([128, 512], F32, name="pq", tag="inj")
                    for ct in range(NCT):
                        nc.tensor.matmul(
                            out=pq[:],
                            lhsT=Wq_sb[:, ct * 512 + g * 128: ct * 512 + (g + 1) * 128],
                            rhs=xcs[s][:, ct * 512:(ct + 1) * 512],
                            start=(ct == 0), stop=(ct == NCT - 1),
                        )
                    nc.scalar.activation(
                        QTs[s][:, g * 512: (g + 1) * 512], pq[:], AF.Identity,
                        bias=bq_sb[:, g:g + 1], scale=SCALE)
                return run

            def k_group(ch, g):
                def run():
                    s = ch % 2
                    pk = inj.tile([128, 512], F32, name="pk", tag="inj")
                    for ct in range(NCT):
                        nc.tensor.matmul(
                            out=pk[:],
                            lhsT=Wk_sb[:, ct * 512 + g * 128: ct * 512 + (g + 1) * 128],
                            rhs=xcs[s][:, ct * 512:(ct + 1) * 512],
                            start=(ct == 0), stop=(ct == NCT - 1),
                        )
                    nc.scalar.activation(
                        KTs[ch][:, g * 512: g * 512 + 512], pk[:], AF.Identity,
                        bias=bk_sb[:, g:g + 1], scale=1.0)
                return run

            def v_group(ch, ts):
                def run():
                    s = ch % 2
                    kt = ch * 4 + ts
                    pv = inj.tile([128, 512], F32, name="pv", tag="inj")
                    for ct in range(NCT):
                        nc.tensor.matmul(
                            out=pv[:],
                            lhsT=xcs[s][:, ct * 512 + ts * 128: ct * 512 + ts * 128 + 128],
                            rhs=Wv_sb[:, ct * 512:(ct + 1) * 512],
                            start=(ct == 0), stop=False,
                        )
                    nc.tensor.matmul(
                        out=pv[:], lhsT=ones128[:], rhs=bv_sb[:],
                        start=False, stop=True,
                    )
                    nc.scalar.copy(Vrc[ch][:, ts, :, 0:64], pv[:])
                return run

            def proj_group(qc, tloc):
                def run():
                    s = qc % 2
                    tt = 4 * qc + tloc
                    for oc in range(2):
                        po = inj.tile([128, 512], F32, name="po", tag="inj")
                        for g in range(4):
                            nc.tensor.matmul(
                                out=po[:],
                                lhsT=YTs[s][:, g * 512 + tloc * 128:
                                            g * 512 + tloc * 128 + 128],
                                rhs=Wp_sb[:, g * C + oc * 512: g * C + oc * 512 + 512],
                                start=(g == 0), stop=(g == 3),
                            )
                        ob = obp.tile([128, 512], F32, name="ob", tag="ob")
                        if oc == 0:
                            nc.scalar.copy(ob[:], po[:])
                        else:
                            nc.vector.tensor_copy(ob[:], po[:])
                        nc.sync.dma_start(
                            out=out_d[tt * 128:(tt + 1) * 128,
                                      oc * 512:(oc + 1) * 512], in_=ob[:])
                return run

            def qk_groups(ch):
                gs = []
                for g in range(4):
                    gs.append(q_group(ch, g))
                    gs.append(k_group(ch, g))
                return gs

            def v_groups(ch):
                return [v_group(ch, ts) for ts in range(4)]

            # ---------------- attention for one (qc, g) ----------------
            def attention_g(qc, g, pump):
                s = qc % 2
                kmax = 4 * qc + 4
                oA = pop.tile([65, 512], F32, name="oA", tag="o")
                oB = pop.tile([65, 512], F32, name="oB", tag="o")
                e_l = [None] * kmax
                off_l = [None] * kmax
                for step in range(kmax + D):
                    if step < kmax:
                        kt = step
                        k0 = kt * 128
                        toff = 128 * (kt - 4 * qc) if kt >= 4 * qc else 0
                        off_l[kt] = toff
                        diag = kt >= 4 * qc
                        sAB = pss.tile([128, 1024], F32, name="sAB", tag="sAB")
                        kch, kloc = kt // 4, kt % 4
                        nc.tensor.matmul(
                            out=sAB[:, toff:512],
                            lhsT=KTs[kch][0:64, g * 512 + kloc * 128:
                                          g * 512 + kloc * 128 + 128],
                            rhs=QTs[s][0:64, g * 512 + toff: (g + 1) * 512],
                            start=True, stop=not diag,
                        )
                        nc.tensor.matmul(
                            out=sAB[:, 512 + toff:1024],
                            lhsT=KTs[kch][64:128, g * 512 + kloc * 128:
                                          g * 512 + kloc * 128 + 128],
                            rhs=QTs[s][64:128, g * 512 + toff: (g + 1) * 512],
                            start=True, stop=not diag, tile_position=(64, 0),
                        )
                        if diag:
                            nc.tensor.matmul(
                                out=sAB[:, toff:toff + 128],
                                lhsT=ident_sb[:], rhs=mask_sb[:],
                                start=False, stop=True,
                            )
                            nc.tensor.matmul(
                                out=sAB[:, 512 + toff:512 + toff + 128],
                                lhsT=ident_sb[:], rhs=mask_sb[:],
                                start=False, stop=True,
                            )
                        eAB = es.tile([128, 1024], FP16, name="eAB", tag="eAB")
                        s3 = sAB.rearrange("p (h w) -> p h w", h=2, w=512)
                        e3 = eAB.rearrange("p (h w) -> p h w", h=2, w=512)
                        nc.scalar.activation(
                            e3[:, :, toff:512], s3[:, :, toff:512], AF.Exp,
                            bias=pad_sb[:, kt:kt + 1])
                        e_l[kt] = eAB
                    pv = step - D
                    if 0 <= pv < kmax:
                        toff = off_l[pv]
                        vch, vbase = pv // 4, (pv % 4) * 520
                        nc.tensor.matmul(
                            out=oA[:, toff:512],
                            lhsT=Vts[vch][:, vbase + 130 * g: vbase + 130 * g + 65],
                            rhs=e_l[pv][:, toff:512],
                            start=(pv == 0), stop=(pv == kmax - 1),
                        )
                        nc.tensor.matmul(
                            out=oB[:, toff:512],
                            lhsT=Vts[vch][:, vbase + 130 * g + 65: vbase + 130 * g + 130],
                            rhs=e_l[pv][:, 512 + toff:1024],
                            start=(pv == 0), stop=(pv == kmax - 1),
                        )
                    pump()
                # epilogue: drain PSUM fast on DVE, normalize off-path
                oa_sb = osb.tile([65, 512], F32, name="oa_sb", tag="osb")
                ob_sb = osb.tile([65, 512], F32, name="ob_sb", tag="osb")
                nc.vector.tensor_copy(oa_sb[:], oA[:])
                nc.vector.tensor_copy(ob_sb[:], oB[:])
                rr = rrs[(qc * 4 + g) % 2]
                rB0 = rB0s[(qc * 4 + g) % 2]
                nc.vector.tensor_copy(rr[0:1, :], oa_sb[64:65, :])
                nc.vector.tensor_copy(rr[32:33, :], ob_sb[64:65, :])
                nc.vector.reciprocal(rr[:], rr[:])
                nc.vector.tensor_copy(rB0[:], rr[32:33, :])
                nc.gpsimd.partition_broadcast(rbA[:], rr[0:1, :])
                nc.gpsimd.partition_broadcast(rbB[:], rB0[:])
                nc.vector.tensor_mul(
                    YTs[s][0:64, g * 512: (g + 1) * 512],
                    oa_sb[0:64, :], rbA[:])
                nc.vector.tensor_mul(
                    YTs[s][64:128, g * 512: (g + 1) * 512],
                    ob_sb[0:64, :], rbB[:])

            # ---------------- the pipelined stream ----------------
            for grp in ([q_group(0, g) for g in range(4)]
                        + [k_group(0, g) for g in range(4)] + v_groups(0)):
                grp()

            order = [0, 1, 3, 2]
            for bi, qc in enumerate(order):
                if bi == 0:
                    load_x(2)
                elif bi == 1:
                    load_x(3)
                early, late, tail = [], [], []
                if qc == 0:
                    late = qk_groups(1) + v_groups(1)
                elif qc == 1:
                    # chunk-3 Q writes QTr slot 1, still read by this block's
                    # S matmuls -> those groups go to `tail`, emitted only
                    # after every qc=1 S matmul is in the stream
                    late = (qk_groups(2) + v_groups(2)
                            + [k_group(3, g) for g in range(4)] + v_groups(3))
                    tail = [q_group(3, g) for g in range(4)]
                elif qc == 3:
                    # proj(1) reads YTr slot 1 which this block's epilogues
                    # overwrite -> emit all proj(1) within g=0's steps
                    early = [proj_group(1, t) for t in range(4)]
                    late = [proj_group(0, t) for t in range(4)]
                else:  # qc == 2
                    late = [proj_group(3, t) for t in range(4)]

                g0_steps = 4 * qc + 4 + D
                nsteps = 4 * g0_steps
                stride_e = max(1, g0_steps // (len(early) + 1))
                stride_l = max(1, nsteps // (len(late) + 1))
                state = {"step": 0}

                def pump(state=state, early=early, late=late,
                         stride_e=stride_e, stride_l=stride_l):
                    state["step"] += 1
                    st = state["step"]
                    if early and st % stride_e == 0:
                        early.pop(0)()
                    elif late and st % stride_l == 0:
                        late.pop(0)()

                for g in range(4):
                    attention_g(qc, g, pump)
                    while early:
                        early.pop(0)()
                while late:
                    late.pop(0)()
                while tail:
                    tail.pop(0)()

            # tail: attention PSUM pools are dead -- use all 8 banks to
            # run proj(2)'s 8 chains g-major so only the final 8 matmuls
            # depend on the last epilogue
            inj.release()
            pop.release()
            pss.release()
            ptail = tc.alloc_tile_pool(name="ptail", bufs=1, space="PSUM")
            combos = [(tloc, oc) for tloc in range(4) for oc in range(2)]
            pos = [ptail.tile([128, 512], F32, name=f"pt{i}", tag=f"pt{i}")
                   for i in range(8)]
            for g in range(4):
                for i, (tloc, oc) in enumerate(combos):
                    nc.tensor.matmul(
                        out=pos[i][:],
                        lhsT=YTs[0][:, g * 512 + tloc * 128:
                                    g * 512 + tloc * 128 + 128],
                        rhs=Wp_sb[:, g * C + oc * 512: g * C + oc * 512 + 512],
                        start=(g == 0), stop=(g == 3),
                    )
            for i, (tloc, oc) in enumerate(combos):
                tt = 8 + tloc
                ob = obp.tile([128, 512], F32, name="ob", tag="ob")
                nc.scalar.copy(ob[:], pos[i][:])
                nc.sync.dma_start(
                    out=out_d[tt * 128:(tt + 1) * 128,
                              oc * 512:(oc + 1) * 512], in_=ob[:])

            for pool in (ptail, obp, osb, es, xs):
                pool.release()

    nc.compile()
    return nc


def _in_maps(x, Wk, bk, Wq, bq, Wv, bv, Wp, bp, padding_mask):
    maps = []
    mask_rows = np.arange(128)[:, None]
    mask_cols = np.arange(128)[None, :]
    maskb = np.where(mask_rows <= mask_cols, 0.0, NEG).astype(np.float32)
    for core in range(8):
        b, half = divmod(core, 2)
        hs = slice(half * IC, (half + 1) * IC)
        maps.append({
            "xT": np.ascontiguousarray(x[b].T).astype(np.float16),
            "WqT": np.ascontiguousarray(Wq[hs, :].T).astype(np.float16),
            "WkT": np.ascontiguousarray(Wk[hs, :].T).astype(np.float16),
            "WvT": np.ascontiguousarray(Wv[hs, :].T).astype(np.float16),
            "WpT": np.ascontiguousarray(Wp[:, hs].T).astype(np.float16),
            "bqs": np.ascontiguousarray((bq[hs] * SCALE).reshape(4, 128).T),
            "bks": np.ascontiguousarray(bk[hs].reshape(4, 128).T),
            "bvr": bv[hs].reshape(1, IC).copy(),
            "padb": np.ascontiguousarray(
                np.where(padding_mask[b] != 0, 0.0, NEG)
                .astype(np.float32).reshape(NKT, 128).T),
            "ones128": np.ones((1, 128), np.float32),
            "ones8": np.ones((128, NKT * 8), np.float16),
            "ident": np.eye(128).astype(ml_dtypes.bfloat16),
            "maskb": maskb.astype(ml_dtypes.bfloat16),
        })
    return maps


def _run(inputs, trace=False, **kw):
    if "nc" not in _CACHE:
        _CACHE["nc"] = _build()
    nc = _CACHE["nc"]
    ins = {k: np.asarray(v, dtype=np.float32) if k != "padding_mask"
           else np.asarray(v) for k, v in inputs.items()}
    maps = _in_maps(**ins)
    res = run_bass_kernel_spmd(nc, maps, core_ids=list(range(8)), trace=trace, **kw)
    bp = np.asarray(inputs["bp"], np.float32)
    y = np.empty((B, T, C), np.float32)
    for b in range(B):
        y[b] = res.results[2 * b]["out"] + res.results[2 * b + 1]["out"] + bp
    return y, res


def kernel(**inputs):
    y, _ = _run(inputs, trace=False)
    return y
